# revision 70
# baseline (speedup 1.0000x reference)
"""BiRWKV layer kernel for 8 Trainium2 NeuronCores.

Strategy (data-parallel over B=8, one batch element per core):
  - (channel, time) layout on chip: channels on the 128 SBUF partitions
    (C=512 -> 4 blocks), time on the free dim.
  - r/k/v projections for both directions are bf16 matmuls
    (lhsT = W block, rhs = x^T block) accumulated over 4 input-channel
    blocks into PSUM (fp32).
  - WKV runs UNSTABILIZED (mathematically equal to the reference's
    log-sum-exp form; values stay in range since |w|*T <= ~28, k~N(0,1)):
        den_t = d*den_{t-1} + e^{k_t};  num_t = d*num_{t-1} + e^{k_t} v_t
        y_t   = (num_{t-1} + e^{k_t+u} v_t) / (den_{t-1} + e^{k_t+u})
  - Both den/num recurrences run on the DVE via 1024-wide
    tensor_tensor_scan (scans are DVE-only; ~2.3ns/col on HW for ANY
    operand dtype/stride mix -- measured; no fast mode exists).
  - Sigmoid is folded into the divisor:
        y = sigmoid(r) * nm / dn = nm / (dn * (1 + e^{-r}))
    so the gate costs one ACT pass (em=exp(-r)) plus one DVE
    scalar_tensor_tensor (dnm = (em+1)*dn); the division is
    rc2 = exp(-ln(dnm)) on ACT (natural_log_exp table; no reloads).
  - Engine split (HW-measured): GpSimd/Pool shares SBUF ports with the
    DVE -- loading Pool beyond ~1 light op inflates every concurrent
    DVE op 30-100% (scans 2.3us->3.1us at 50% Pool duty), so ALL wide
    elementwise runs on the DVE and Pool is left idle.  ekb=e^u*ek and
    ekbv=e^u*ekv run on ACT as Copy-with-per-partition-scale; ACT also
    does ek/em (exp) and ln/rc2 (division).  DVE keeps scans, ekv
    (psum read), dn/nm adds, dnm stt, and the y mul.  Keep all DVE
    operands flat-2D bf16: 3-D/strided APs measurably slow scans+stt.
  - Scan chaining uses persistent full-T buffers [128, T+1] per
    (direction-kind, cb): pair p's carry column is adjacent to pair
    p+1's output region by construction, so the scan `initial` reads
    the previous output directly -- zero carry copies or memsets in
    the steady state (was 48 copies + latency on the scan chain).
  - Emission order a(p) b(p) c(p) is load-bearing: both ACT and DVE
    queues are in-order, so hoisting a(p+1) ahead of b(p) (or merging
    b into a) head-of-line-blocks one queue on the other's late
    dependencies -- all such reorderings measured slower.
  - Output projection is SPLIT across phases to level PE load:
    fwd phase computes y_f @ Wout_top, ACT-copies psum to bf16 staging
    and DMAs it to a DRAM scratch; bwd phase DMAs it back and re-seeds
    psum via an identity matmul (DMA cannot touch PSUM), accumulates
    y_b @ Wout_bot with start=False matmuls, then copies out as bf16.
  - PSUM tags: k/r projections 4 bufs, v 2 (DVE consumes v late),
    part_c pso 2 -- exactly the 8 banks.
  - PE p-state: the clock resets to 1.2GHz after >~100ns idle and
    needs ~3us continuous work to re-reach 2.4GHz; junk warm-up
    matmuls bridge each phase's final gate-chain idle (partial fix --
    per-m-tile y-dependency stalls still re-drop the clock).
  - Startup: w_kf/w_vf + packed [C,4] constants (eu_f,eu_b,dec_f,dec_b)
    are DMA'd first; w_rf + bwd weights + Wout issue after the first
    pair's x tiles (sync-queue DMA order is FIFO and is the startup
    critical path at ~600ns per descriptor issue).
"""

import numpy as np
import ml_dtypes

B, T, C = 8, 4096, 512
TT = 512           # time tile (psum width)
CB = 4             # channel blocks
PW = 2 * TT        # pair width for SBUF-side elementwise
NP = T // PW       # 4 pairs

_CACHE = {}


def _apply_tile_patches():
    """walrus in this container rejects instructions with >1 sync wait
    ("Too many sync wait commands"). Split excess waits onto same-engine
    nop carriers, and do the same for the TileContext tail drain."""
    import concourse.tile as tile_mod
    from concourse import mybir
    from concourse.vector_clock import ScopedClock

    if getattr(tile_mod, "_wait_split_patched", False):
        return
    MAXW = 1

    _orig_add = tile_mod.TileContext._add_instruction

    def _split_add(self, inst):
        si = inst.sync_info
        if si is not None and si.on_wait and len(si.on_wait) > MAXW:
            waits = list(si.on_wait)
            k = 0
            while len(waits) > MAXW:
                chunk, waits = waits[:MAXW], waits[MAXW:]
                carrier = mybir.InstNoOp(
                    name=f"{inst.name}_wsplit{k}",
                    engine=inst.engine,
                    bass_nofuse=True,
                    sync_info=mybir.SyncInfo(on_wait=chunk, on_update=[]),
                )
                k += 1
                _orig_add(self, carrier)
            inst.sync_info = mybir.SyncInfo(
                on_wait=waits, on_update=list(si.on_update)
            )
        return _orig_add(self, inst)

    def _drain_and_barrier(self, tick_clock, wait_clock):
        drain_inst = self.nc.sync.drain()
        wait_clock.add_sem_waits(
            drain_inst.ins, ScopedClock({None: tick_clock.global_clock})
        )
        si = drain_inst.ins.sync_info
        if si is not None and si.on_wait and len(si.on_wait) > MAXW:
            waits = list(si.on_wait)
            drain_inst.ins.sync_info = mybir.SyncInfo(
                on_wait=waits[:MAXW], on_update=list(si.on_update)
            )
            rest = waits[MAXW:]
            while rest:
                chunk, rest = rest[:MAXW], rest[MAXW:]
                n = self.nc.sync.nop(nofuse=True)
                n.ins.sync_info = mybir.SyncInfo(on_wait=chunk, on_update=[])

        self.nc.all_engine_barrier()
        assert self.sems is not None
        popped = self.nc._tile_sem_poison_stack.pop()
        assert popped is self._sem_poison
        self.nc.clear_and_free_semaphores(list(self.sems.allocated().values()))
        self.nc.all_engine_barrier()

    tile_mod.TileContext._add_instruction = _split_add
    tile_mod.TileContext._drain_and_barrier = _drain_and_barrier
    tile_mod._wait_split_patched = True


def _build_nc():
    import concourse.bass as bass
    import concourse.tile as tile
    from concourse import mybir

    _apply_tile_patches()

    f32 = mybir.dt.float32
    bf16 = mybir.dt.bfloat16
    Alu = mybir.AluOpType
    Act = mybir.ActivationFunctionType

    nc = bass.Bass()

    xT = nc.dram_tensor("xT", [C, T], bf16, kind="ExternalInput")
    wnames = ["w_rf", "w_kf", "w_vf", "w_rb", "w_kb", "w_vb"]
    wdram = {
        n: nc.dram_tensor(n, [128, 4 * C], bf16, kind="ExternalInput")
        for n in wnames
    }
    wout_d = nc.dram_tensor("wout", [128, 8 * C], bf16, kind="ExternalInput")
    # packed constants: columns = eu_f, eu_b, dec_f, dec_b
    cvec_d = nc.dram_tensor("cvec", [C, 4], f32, kind="ExternalInput")
    ident_d = nc.dram_tensor("ident", [128, 128], bf16, kind="ExternalInput")
    part_d = nc.dram_tensor("part", [T, C], bf16, kind="Internal")
    out_d = nc.dram_tensor("y", [T, C], bf16, kind="ExternalOutput")

    def act(*args, **kwargs):
        return nc.scalar.activation(*args, **kwargs)

    with tile.TileContext(nc) as tc:
        with (
            tc.tile_pool(name="wp", bufs=1) as wp,
            tc.tile_pool(name="cst", bufs=1) as cst,
            tc.tile_pool(name="ypf", bufs=1) as ypfp,
            tc.tile_pool(name="chain", bufs=2) as chainp,
            tc.tile_pool(name="xt", bufs=2) as xtp,
            tc.tile_pool(name="wk", bufs=1) as wkp,
            tc.tile_pool(name="ps", bufs=1, space="PSUM") as psp,
        ):
            # ---- resident weights & constants (fwd-needed first) ----
            wt = {}
            for n in wnames:
                wt[n] = wp.tile([128, 4 * C], bf16, tag=n, name=n)
            wout = wp.tile([128, 8 * C], bf16, name="wout")
            for n in ("w_kf", "w_vf"):
                nc.sync.dma_start(wt[n][:], wdram[n][:])
            cvt = {}
            for cb in range(CB):
                sl = slice(cb * 128, (cb + 1) * 128)
                cvt[cb] = cst.tile([128, 4], f32, tag=f"cv{cb}",
                                   name=f"cv{cb}")
                nc.sync.dma_start(cvt[cb][:], cvec_d[sl, :])
            ident = cst.tile([128, 128], bf16, tag="ident", name="ident")
            nc.sync.dma_start(ident[:], ident_d[:])

            deferred_done = [False]

            def dma_deferred():
                if deferred_done[0]:
                    return
                deferred_done[0] = True
                for n in ("w_rf", "w_kb", "w_vb", "w_rb"):
                    nc.sync.dma_start(wt[n][:], wdram[n][:])
                nc.sync.dma_start(wout[:], wout_d[:])

            # forward-direction y, resident in SBUF across both phases
            ypf = {}
            for cb in range(CB):
                ypf[cb] = ypfp.tile([128, T], bf16, tag=f"ypf{cb}",
                                    name=f"ypf{cb}")

            def run_phase(d):
                fwd = d == "f"
                eucol = 0 if fwd else 1
                deccol = 2 if fwd else 3
                wr, wk, wv = wt["w_r" + d], wt["w_k" + d], wt["w_v" + d]
                pairs = list(range(NP)) if fwd else list(reversed(range(NP)))
                # persistent full-T scan buffers: pair p's carry column is
                # adjacent to pair p+1's output region by construction, so
                # chaining needs no copies -- `initial` points straight at
                # the previous pair's last output column.
                chd, chn = {}, {}
                for cb in range(CB):
                    chd[cb] = chainp.tile([128, T + 1], bf16, bufs=1,
                                          tag=f"chd{cb}", name=f"chd{cb}")
                    chn[cb] = chainp.tile([128, T + 1], bf16, bufs=1,
                                          tag=f"chn{cb}", name=f"chn{cb}")
                    if fwd:
                        nc.vector.memset(chd[cb][:, 0:1], 0.0)
                        nc.vector.memset(chn[cb][:, 0:1], 0.0)
                    else:
                        nc.vector.memset(chd[cb][:, T: T + 1], 0.0)
                        nc.vector.memset(chn[cb][:, T: T + 1], 0.0)

                stash = {}     # (pr, cb) -> (dnm, nm)
                ypb_tiles = {}  # (pr, cb) -> y tile (bwd only)

                def part_a(pr, first=False):
                    p0 = pr * PW
                    xts = {}
                    for half, tt in enumerate((2 * pr, 2 * pr + 1)):
                        t0 = tt * TT
                        for kb in range(4):
                            xt = xtp.tile([128, TT], bf16,
                                          tag=f"xt{kb}h{half}",
                                          bufs=2, name=f"xt{kb}h{half}")
                            nc.sync.dma_start(
                                xt[:],
                                xT[kb * 128:(kb + 1) * 128, t0: t0 + TT])
                            xts[(half, kb)] = xt
                    if first:
                        dma_deferred()
                    for cb in range(CB):
                        eu = cvt[cb][:, eucol:eucol + 1]
                        decbc = cvt[cb][:, deccol:deccol + 1].broadcast_to(
                            [128, PW])
                        pss = {}
                        for cls, w in (("k", wk), ("v", wv), ("r", wr)):
                            tag, nb = (("psv", 2) if cls == "v"
                                       else ("ps", 4))
                            for half in range(2):
                                pss[(cls, half)] = psp.tile(
                                    [128, TT], f32, tag=tag, bufs=nb,
                                    name=f"ps{cls}")
                            for kb in range(4):
                                wsl = w[:, kb * C + cb * 128:
                                        kb * C + cb * 128 + 128]
                                for half in range(2):
                                    nc.tensor.matmul(
                                        pss[(cls, half)][:], wsl,
                                        xts[(half, kb)][:],
                                        start=(kb == 0), stop=(kb == 3))
                        ek = wkp.tile([128, PW], bf16, tag="ek", bufs=2,
                                      name="ek")
                        em = wkp.tile([128, PW], bf16, tag="em", bufs=2,
                                      name="em")
                        ekv = wkp.tile([128, PW], bf16, tag="ekv", bufs=2,
                                       name="ekv")
                        for half in range(2):
                            hs = slice(half * TT, (half + 1) * TT)
                            act(ek[:, hs], pss[("k", half)][:], Act.Exp)
                            act(em[:, hs], pss[("r", half)][:], Act.Exp,
                                bias=0.0, scale=-1.0)
                            nc.vector.tensor_mul(ekv[:, hs], ek[:, hs],
                                                 pss[("v", half)][:])
                        ekb = wkp.tile([128, PW], bf16, tag="ekb", bufs=2,
                                       name="ekb")
                        act(ekb[:], ek[:], Act.Copy, bias=0.0, scale=eu)
                        ekbv = wkp.tile([128, PW], bf16, tag="ekbv", bufs=2,
                                        name="ekbv")
                        act(ekbv[:], ekv[:], Act.Copy, bias=0.0, scale=eu)
                        denb, numb = chd[cb], chn[cb]
                        if fwd:
                            nc.vector.tensor_tensor_scan(
                                denb[:, p0 + 1: p0 + 1 + PW], decbc, ek[:],
                                denb[:, p0: p0 + 1], Alu.mult, Alu.add)
                            nc.vector.tensor_tensor_scan(
                                numb[:, p0 + 1: p0 + 1 + PW], decbc, ekv[:],
                                numb[:, p0: p0 + 1], Alu.mult, Alu.add)
                            den_prev = denb[:, p0: p0 + PW]
                            num_prev = numb[:, p0: p0 + PW]
                        else:
                            nc.vector.tensor_tensor_scan(
                                denb[:, p0: p0 + PW][:, ::-1], decbc,
                                ek[:][:, ::-1],
                                denb[:, p0 + PW: p0 + PW + 1],
                                Alu.mult, Alu.add)
                            nc.vector.tensor_tensor_scan(
                                numb[:, p0: p0 + PW][:, ::-1], decbc,
                                ekv[:][:, ::-1],
                                numb[:, p0 + PW: p0 + PW + 1],
                                Alu.mult, Alu.add)
                            den_prev = denb[:, p0 + 1: p0 + 1 + PW]
                            num_prev = numb[:, p0 + 1: p0 + 1 + PW]
                        dn = wkp.tile([128, PW], bf16, tag="dn", bufs=2,
                                      name="dn")
                        nm = wkp.tile([128, PW], bf16, tag="nm", bufs=4,
                                      name="nm")
                        nc.vector.tensor_add(dn[:], ekb[:], den_prev)
                        nc.vector.tensor_add(nm[:], ekbv[:], num_prev)
                        dnm = wkp.tile([128, PW], bf16, tag="dnm", bufs=4,
                                       name="dnm")
                        nc.vector.scalar_tensor_tensor(
                            dnm[:], em[:], 1.0, dn[:], Alu.add, Alu.mult)
                        stash[(pr, cb)] = (dnm, nm)

                def part_b(pr, warm=False):
                    p0 = pr * PW
                    if warm:
                        # keep the PE clock from p-state-resetting during
                        # the final gate chain: ~8 junk matmuls on resident
                        # data bridge the idle window so the phase-tail
                        # projections run at full clock (605ns -> 379ns)
                        jp = psp.tile([128, C], f32, tag="pso", bufs=2,
                                      name="jwarm")
                        for _ in range(8):
                            nc.tensor.matmul(jp[:], wout[:, 0:128],
                                             wout[:, 0:C],
                                             start=True, stop=True)
                    for cb in range(CB):
                        dnm, nm = stash.pop((pr, cb))
                        ln = wkp.tile([128, PW], f32, tag="ln", bufs=1,
                                      name="ln")
                        act(ln[:], dnm[:], Act.Ln)
                        rc2 = wkp.tile([128, PW], bf16, tag="rc2", bufs=2,
                                       name="rc2")
                        act(rc2[:], ln[:], Act.Exp, scale=-1.0)
                        if fwd:
                            nc.vector.tensor_mul(
                                ypf[cb][:, p0: p0 + PW], nm[:], rc2[:])
                        else:
                            yb = wkp.tile([128, PW], bf16, tag=f"ypb{cb}",
                                          bufs=1, name=f"ypb{cb}")
                            nc.vector.tensor_mul(yb[:], nm[:], rc2[:])
                            ypb_tiles[(pr, cb)] = yb

                def part_c(pr, last=False):
                    p0 = pr * PW
                    for m in range(PW // 128):
                        t0 = p0 + m * 128
                        pso = psp.tile([128, C], f32, tag="pso", bufs=2,
                                       name="pso")
                        if fwd:
                            for cb in range(CB):
                                nc.tensor.matmul(
                                    pso[:],
                                    ypf[cb][:, t0: t0 + 128],
                                    wout[:, cb * C: (cb + 1) * C],
                                    start=(cb == 0), stop=(cb == 3))
                            pstg = wkp.tile([128, C], bf16, tag="pstg",
                                            bufs=3, name="pstg")
                            nc.scalar.copy(pstg[:], pso[:])
                            nc.sync.dma_start(part_d[t0: t0 + 128, :],
                                              pstg[:])
                        else:
                            pstg = wkp.tile([128, C], bf16, tag="pstg",
                                            bufs=3, name="pstg")
                            nc.sync.dma_start(pstg[:],
                                              part_d[t0: t0 + 128, :])
                            nc.tensor.matmul(
                                pso[:], ident[:], pstg[:],
                                start=True, stop=False)
                            for cb in range(CB):
                                nc.tensor.matmul(
                                    pso[:],
                                    ypb_tiles[(pr, cb)][:, m * 128:
                                                        (m + 1) * 128],
                                    wout[:, (4 + cb) * C: (5 + cb) * C],
                                    start=False, stop=(cb == 3))
                            osb = wkp.tile([128, C], bf16, tag="osb",
                                           bufs=2, name="osb")
                            if last and m % 2 == 0:
                                nc.vector.tensor_copy(osb[:], pso[:])
                            else:
                                nc.scalar.copy(osb[:], pso[:])
                            nc.sync.dma_start(out_d[t0: t0 + 128, :],
                                              osb[:])

                for i, pr in enumerate(pairs):
                    part_a(pr, first=(fwd and i == 0))
                    part_b(pr, warm=(i == len(pairs) - 1))
                    part_c(pr, last=(i == len(pairs) - 1))
                    if not fwd:
                        for key in list(ypb_tiles):
                            if key[0] == pr:
                                del ypb_tiles[key]

            run_phase("f")
            run_phase("b")

    return nc


def _host_prep(x, W_rkv, W_out, time_decay, time_first, time_decay_rev,
               time_first_rev):
    bf16 = ml_dtypes.bfloat16
    f32 = np.float32

    Wr = W_rkv.reshape(C, 2, 3, C)
    pieces = {
        "w_rf": Wr[:, 0, 0], "w_kf": Wr[:, 0, 1], "w_vf": Wr[:, 0, 2],
        "w_rb": Wr[:, 1, 0], "w_kb": Wr[:, 1, 1], "w_vb": Wr[:, 1, 2],
    }
    wmaps = {}
    for n, p in pieces.items():
        wmaps[n] = np.ascontiguousarray(
            p.reshape(4, 128, C).transpose(1, 0, 2).reshape(128, 4 * C)
        ).astype(bf16)

    Wo = W_out.reshape(8, 128, C).transpose(1, 0, 2).reshape(128, 8 * C)
    wout = np.ascontiguousarray(Wo).astype(bf16)

    eu_f = np.exp(time_first.astype(np.float64)).reshape(C)
    eu_b = np.exp(time_first_rev.astype(np.float64)).reshape(C)
    dec_f = np.exp(-np.exp(time_decay.astype(np.float64))).reshape(C)
    dec_b = np.exp(-np.exp(time_decay_rev.astype(np.float64))).reshape(C)
    cvec = np.ascontiguousarray(
        np.stack([eu_f, eu_b, dec_f, dec_b], axis=1)
    ).astype(f32)

    ident = np.eye(128, dtype=np.float32).astype(bf16)
    shared = dict(wout=wout, cvec=cvec, ident=ident, **wmaps)
    in_maps = []
    for b in range(B):
        m = dict(shared)
        m["xT"] = np.ascontiguousarray(x[b].T).astype(bf16)
        in_maps.append(m)
    return in_maps


def kernel(x, W_rkv, W_out, time_decay, time_first, time_decay_rev,
           time_first_rev, _trace=False):
    from concourse.bass_utils import run_bass_kernel_spmd

    x = np.asarray(x, dtype=np.float32)
    W_rkv = np.asarray(W_rkv, dtype=np.float32)
    W_out = np.asarray(W_out, dtype=np.float32)
    time_decay = np.asarray(time_decay, dtype=np.float32)
    time_first = np.asarray(time_first, dtype=np.float32)
    time_decay_rev = np.asarray(time_decay_rev, dtype=np.float32)
    time_first_rev = np.asarray(time_first_rev, dtype=np.float32)

    if "nc" not in _CACHE:
        _CACHE["nc"] = _build_nc()
    nc = _CACHE["nc"]

    in_maps = _host_prep(x, W_rkv, W_out, time_decay, time_first,
                         time_decay_rev, time_first_rev)
    res = run_bass_kernel_spmd(
        nc, in_maps, core_ids=list(range(B)), trace=_trace
    )
    _CACHE["last_result"] = res
    out = np.stack([res.results[b]["y"].astype(np.float32) for b in range(B)])
    return out


# revision 71
# speedup vs baseline: 1.0005x; 1.0005x over previous
"""BiRWKV layer kernel for 8 Trainium2 NeuronCores.

Strategy (data-parallel over B=8, one batch element per core):
  - (channel, time) layout on chip: channels on the 128 SBUF partitions
    (C=512 -> 4 blocks), time on the free dim.
  - r/k/v projections for both directions are bf16 matmuls
    (lhsT = W block, rhs = x^T block) accumulated over 4 input-channel
    blocks into PSUM (fp32).
  - WKV runs UNSTABILIZED (mathematically equal to the reference's
    log-sum-exp form; values stay in range since |w|*T <= ~28, k~N(0,1)):
        den_t = d*den_{t-1} + e^{k_t};  num_t = d*num_{t-1} + e^{k_t} v_t
        y_t   = (num_{t-1} + e^{k_t+u} v_t) / (den_{t-1} + e^{k_t+u})
  - Both den/num recurrences run on the DVE via 1024-wide
    tensor_tensor_scan (scans are DVE-only; ~2.3ns/col on HW for ANY
    operand dtype/stride mix -- measured; no fast mode exists).
  - Sigmoid is folded into the divisor:
        y = sigmoid(r) * nm / dn = nm / (dn * (1 + e^{-r}))
    so the gate costs one ACT pass (em=exp(-r)) plus one DVE
    scalar_tensor_tensor (dnm = (em+1)*dn); the division is
    rc2 = exp(-ln(dnm)) on ACT (natural_log_exp table; no reloads).
  - Engine split (HW-measured): GpSimd/Pool shares SBUF ports with the
    DVE -- loading Pool beyond ~1 light op inflates every concurrent
    DVE op 30-100% (scans 2.3us->3.1us at 50% Pool duty), so ALL wide
    elementwise runs on the DVE and Pool is left idle.  ekb=e^u*ek and
    ekbv=e^u*ekv run on ACT as Copy-with-per-partition-scale; ACT also
    does ek/em (exp) and ln/rc2 (division).  DVE keeps scans, ekv
    (psum read), dn/nm adds, dnm stt, and the y mul.  Keep all DVE
    operands flat-2D bf16: 3-D/strided APs measurably slow scans+stt.
  - Scan chaining uses persistent full-T buffers [128, T+1] per
    (direction-kind, cb): pair p's carry column is adjacent to pair
    p+1's output region by construction, so the scan `initial` reads
    the previous output directly -- zero carry copies or memsets in
    the steady state (was 48 copies + latency on the scan chain).
  - Emission order a(p) b(p) c(p) is load-bearing: both ACT and DVE
    queues are in-order, so hoisting a(p+1) ahead of b(p) (or merging
    b into a) head-of-line-blocks one queue on the other's late
    dependencies -- all such reorderings measured slower.
  - Output projection is SPLIT across phases to level PE load:
    fwd phase computes y_f @ Wout_top, ACT-copies psum to bf16 staging
    and DMAs it to a DRAM scratch; bwd phase DMAs it back and re-seeds
    psum via an identity matmul (DMA cannot touch PSUM), accumulates
    y_b @ Wout_bot with start=False matmuls, then copies out as bf16.
  - PSUM tags: k/r projections 4 bufs, v 2 (DVE consumes v late),
    part_c pso 2 -- exactly the 8 banks.
  - PE p-state: the clock resets to 1.2GHz after >~100ns idle and
    needs ~3us continuous work to re-reach 2.4GHz; junk warm-up
    matmuls bridge each phase's final gate-chain idle (partial fix --
    per-m-tile y-dependency stalls still re-drop the clock).
  - Startup: w_kf/w_vf + packed [C,4] constants (eu_f,eu_b,dec_f,dec_b)
    are DMA'd first; w_rf + bwd weights + Wout issue after the first
    pair's x tiles (sync-queue DMA order is FIFO and is the startup
    critical path at ~600ns per descriptor issue).
"""

import numpy as np
import ml_dtypes
#
#

B, T, C = 8, 4096, 512
TT = 512           # time tile (psum width)
CB = 4             # channel blocks
PW = 2 * TT        # pair width for SBUF-side elementwise
NP = T // PW       # 4 pairs

_CACHE = {}


def _apply_tile_patches():
    """walrus in this container rejects instructions with >1 sync wait
    ("Too many sync wait commands"). Split excess waits onto same-engine
    nop carriers, and do the same for the TileContext tail drain."""
    import concourse.tile as tile_mod
    from concourse import mybir
    from concourse.vector_clock import ScopedClock

    if getattr(tile_mod, "_wait_split_patched", False):
        return
    MAXW = 1

    _orig_add = tile_mod.TileContext._add_instruction

    def _split_add(self, inst):
        si = inst.sync_info
        if si is not None and si.on_wait and len(si.on_wait) > MAXW:
            waits = list(si.on_wait)
            k = 0
            while len(waits) > MAXW:
                chunk, waits = waits[:MAXW], waits[MAXW:]
                carrier = mybir.InstNoOp(
                    name=f"{inst.name}_wsplit{k}",
                    engine=inst.engine,
                    bass_nofuse=True,
                    sync_info=mybir.SyncInfo(on_wait=chunk, on_update=[]),
                )
                k += 1
                _orig_add(self, carrier)
            inst.sync_info = mybir.SyncInfo(
                on_wait=waits, on_update=list(si.on_update)
            )
        return _orig_add(self, inst)

    def _drain_and_barrier(self, tick_clock, wait_clock):
        drain_inst = self.nc.sync.drain()
        wait_clock.add_sem_waits(
            drain_inst.ins, ScopedClock({None: tick_clock.global_clock})
        )
        si = drain_inst.ins.sync_info
        if si is not None and si.on_wait and len(si.on_wait) > MAXW:
            waits = list(si.on_wait)
            drain_inst.ins.sync_info = mybir.SyncInfo(
                on_wait=waits[:MAXW], on_update=list(si.on_update)
            )
            rest = waits[MAXW:]
            while rest:
                chunk, rest = rest[:MAXW], rest[MAXW:]
                n = self.nc.sync.nop(nofuse=True)
                n.ins.sync_info = mybir.SyncInfo(on_wait=chunk, on_update=[])

        self.nc.all_engine_barrier()
        assert self.sems is not None
        popped = self.nc._tile_sem_poison_stack.pop()
        assert popped is self._sem_poison
        self.nc.clear_and_free_semaphores(list(self.sems.allocated().values()))
        self.nc.all_engine_barrier()

    tile_mod.TileContext._add_instruction = _split_add
    tile_mod.TileContext._drain_and_barrier = _drain_and_barrier
    tile_mod._wait_split_patched = True


def _build_nc():
    import concourse.bass as bass
    import concourse.tile as tile
    from concourse import mybir

    _apply_tile_patches()

    f32 = mybir.dt.float32
    bf16 = mybir.dt.bfloat16
    Alu = mybir.AluOpType
    Act = mybir.ActivationFunctionType

    nc = bass.Bass()

    xT = nc.dram_tensor("xT", [C, T], bf16, kind="ExternalInput")
    wnames = ["w_rf", "w_kf", "w_vf", "w_rb", "w_kb", "w_vb"]
    wdram = {
        n: nc.dram_tensor(n, [128, 4 * C], bf16, kind="ExternalInput")
        for n in wnames
    }
    wout_d = nc.dram_tensor("wout", [128, 8 * C], bf16, kind="ExternalInput")
    # packed constants: columns = eu_f, eu_b, dec_f, dec_b
    cvec_d = nc.dram_tensor("cvec", [C, 4], f32, kind="ExternalInput")
    ident_d = nc.dram_tensor("ident", [128, 128], bf16, kind="ExternalInput")
    part_d = nc.dram_tensor("part", [T, C], bf16, kind="Internal")
    out_d = nc.dram_tensor("y", [T, C], bf16, kind="ExternalOutput")

    def act(*args, **kwargs):
        return nc.scalar.activation(*args, **kwargs)

    with tile.TileContext(nc) as tc:
        with (
            tc.tile_pool(name="wp", bufs=1) as wp,
            tc.tile_pool(name="cst", bufs=1) as cst,
            tc.tile_pool(name="ypf", bufs=1) as ypfp,
            tc.tile_pool(name="chain", bufs=2) as chainp,
            tc.tile_pool(name="xt", bufs=2) as xtp,
            tc.tile_pool(name="wk", bufs=1) as wkp,
            tc.tile_pool(name="ps", bufs=1, space="PSUM") as psp,
        ):
            # ---- resident weights & constants (fwd-needed first) ----
            wt = {}
            for n in wnames:
                wt[n] = wp.tile([128, 4 * C], bf16, tag=n, name=n)
            wout = wp.tile([128, 8 * C], bf16, name="wout")
            for n in ("w_kf", "w_vf"):
                nc.sync.dma_start(wt[n][:], wdram[n][:])
            cvt = {}
            for cb in range(CB):
                sl = slice(cb * 128, (cb + 1) * 128)
                cvt[cb] = cst.tile([128, 4], f32, tag=f"cv{cb}",
                                   name=f"cv{cb}")
                nc.sync.dma_start(cvt[cb][:], cvec_d[sl, :])
            ident = cst.tile([128, 128], bf16, tag="ident", name="ident")
            nc.sync.dma_start(ident[:], ident_d[:])

            deferred_done = [False]

            def dma_deferred():
                if deferred_done[0]:
                    return
                deferred_done[0] = True
                for n in ("w_rf", "w_kb", "w_vb", "w_rb"):
                    nc.sync.dma_start(wt[n][:], wdram[n][:])
                nc.sync.dma_start(wout[:], wout_d[:])

            # forward-direction y, resident in SBUF across both phases
            ypf = {}
            for cb in range(CB):
                ypf[cb] = ypfp.tile([128, T], bf16, tag=f"ypf{cb}",
                                    name=f"ypf{cb}")

            def run_phase(d):
                fwd = d == "f"
                eucol = 0 if fwd else 1
                deccol = 2 if fwd else 3
                wr, wk, wv = wt["w_r" + d], wt["w_k" + d], wt["w_v" + d]
                pairs = list(range(NP)) if fwd else list(reversed(range(NP)))
                # persistent full-T scan buffers: pair p's carry column is
                # adjacent to pair p+1's output region by construction, so
                # chaining needs no copies -- `initial` points straight at
                # the previous pair's last output column.
                chd, chn = {}, {}
                for cb in range(CB):
                    chd[cb] = chainp.tile([128, T + 1], bf16, bufs=1,
                                          tag=f"chd{cb}", name=f"chd{cb}")
                    chn[cb] = chainp.tile([128, T + 1], bf16, bufs=1,
                                          tag=f"chn{cb}", name=f"chn{cb}")
                    if fwd:
                        nc.vector.memset(chd[cb][:, 0:1], 0.0)
                        nc.vector.memset(chn[cb][:, 0:1], 0.0)
                    else:
                        nc.vector.memset(chd[cb][:, T: T + 1], 0.0)
                        nc.vector.memset(chn[cb][:, T: T + 1], 0.0)

                stash = {}     # (pr, cb) -> (dnm, nm)
                ypb_tiles = {}  # (pr, cb) -> y tile (bwd only)

                def part_a(pr, first=False):
                    p0 = pr * PW
                    xts = {}
                    for half, tt in enumerate((2 * pr, 2 * pr + 1)):
                        t0 = tt * TT
                        for kb in range(4):
                            xt = xtp.tile([128, TT], bf16,
                                          tag=f"xt{kb}h{half}",
                                          bufs=2, name=f"xt{kb}h{half}")
                            nc.sync.dma_start(
                                xt[:],
                                xT[kb * 128:(kb + 1) * 128, t0: t0 + TT])
                            xts[(half, kb)] = xt
                    if first:
                        dma_deferred()
                    for cb in range(CB):
                        eu = cvt[cb][:, eucol:eucol + 1]
                        decbc = cvt[cb][:, deccol:deccol + 1].broadcast_to(
                            [128, PW])
                        pss = {}
                        for cls, w in (("k", wk), ("v", wv), ("r", wr)):
                            tag, nb = (("psv", 2) if cls == "v"
                                       else ("ps", 4))
                            for half in range(2):
                                pss[(cls, half)] = psp.tile(
                                    [128, TT], f32, tag=tag, bufs=nb,
                                    name=f"ps{cls}")
                            for kb in range(4):
                                wsl = w[:, kb * C + cb * 128:
                                        kb * C + cb * 128 + 128]
                                for half in range(2):
                                    nc.tensor.matmul(
                                        pss[(cls, half)][:], wsl,
                                        xts[(half, kb)][:],
                                        start=(kb == 0), stop=(kb == 3))
                        ek = wkp.tile([128, PW], bf16, tag="ek", bufs=2,
                                      name="ek")
                        em = wkp.tile([128, PW], bf16, tag="em", bufs=2,
                                      name="em")
                        ekv = wkp.tile([128, PW], bf16, tag="ekv", bufs=2,
                                       name="ekv")
                        for half in range(2):
                            hs = slice(half * TT, (half + 1) * TT)
                            act(ek[:, hs], pss[("k", half)][:], Act.Exp)
                            act(em[:, hs], pss[("r", half)][:], Act.Exp,
                                bias=0.0, scale=-1.0)
                            nc.vector.tensor_mul(ekv[:, hs], ek[:, hs],
                                                 pss[("v", half)][:])
                        ekb = wkp.tile([128, PW], bf16, tag="ekb", bufs=2,
                                       name="ekb")
                        act(ekb[:], ek[:], Act.Copy, bias=0.0, scale=eu)
                        ekbv = wkp.tile([128, PW], bf16, tag="ekbv", bufs=2,
                                        name="ekbv")
                        act(ekbv[:], ekv[:], Act.Copy, bias=0.0, scale=eu)
                        denb, numb = chd[cb], chn[cb]
                        if fwd:
                            nc.vector.tensor_tensor_scan(
                                denb[:, p0 + 1: p0 + 1 + PW], decbc, ek[:],
                                denb[:, p0: p0 + 1], Alu.mult, Alu.add)
                            nc.vector.tensor_tensor_scan(
                                numb[:, p0 + 1: p0 + 1 + PW], decbc, ekv[:],
                                numb[:, p0: p0 + 1], Alu.mult, Alu.add)
                            den_prev = denb[:, p0: p0 + PW]
                            num_prev = numb[:, p0: p0 + PW]
                        else:
                            nc.vector.tensor_tensor_scan(
                                denb[:, p0: p0 + PW][:, ::-1], decbc,
                                ek[:][:, ::-1],
                                denb[:, p0 + PW: p0 + PW + 1],
                                Alu.mult, Alu.add)
                            nc.vector.tensor_tensor_scan(
                                numb[:, p0: p0 + PW][:, ::-1], decbc,
                                ekv[:][:, ::-1],
                                numb[:, p0 + PW: p0 + PW + 1],
                                Alu.mult, Alu.add)
                            den_prev = denb[:, p0 + 1: p0 + 1 + PW]
                            num_prev = numb[:, p0 + 1: p0 + 1 + PW]
                        dn = wkp.tile([128, PW], bf16, tag="dn", bufs=2,
                                      name="dn")
                        nm = wkp.tile([128, PW], bf16, tag="nm", bufs=4,
                                      name="nm")
                        nc.vector.tensor_add(dn[:], ekb[:], den_prev)
                        nc.vector.tensor_add(nm[:], ekbv[:], num_prev)
                        dnm = wkp.tile([128, PW], bf16, tag="dnm", bufs=4,
                                       name="dnm")
                        nc.vector.scalar_tensor_tensor(
                            dnm[:], em[:], 1.0, dn[:], Alu.add, Alu.mult)
                        stash[(pr, cb)] = (dnm, nm)

                def part_b(pr, warm=False):
                    p0 = pr * PW
                    if warm:
                        # keep the PE clock from p-state-resetting during
                        # the final gate chain: ~8 junk matmuls on resident
                        # data bridge the idle window so the phase-tail
                        # projections run at full clock (605ns -> 379ns)
                        jp = psp.tile([128, C], f32, tag="pso", bufs=2,
                                      name="jwarm")
                        for _ in range(8):
                            nc.tensor.matmul(jp[:], wout[:, 0:128],
                                             wout[:, 0:C],
                                             start=True, stop=True)
                    for cb in range(CB):
                        dnm, nm = stash.pop((pr, cb))
                        ln = wkp.tile([128, PW], f32, tag="ln", bufs=1,
                                      name="ln")
                        act(ln[:], dnm[:], Act.Ln)
                        rc2 = wkp.tile([128, PW], bf16, tag="rc2", bufs=2,
                                       name="rc2")
                        act(rc2[:], ln[:], Act.Exp, scale=-1.0)
                        if fwd:
                            nc.vector.tensor_mul(
                                ypf[cb][:, p0: p0 + PW], nm[:], rc2[:])
                        else:
                            yb = wkp.tile([128, PW], bf16, tag=f"ypb{cb}",
                                          bufs=1, name=f"ypb{cb}")
                            nc.vector.tensor_mul(yb[:], nm[:], rc2[:])
                            ypb_tiles[(pr, cb)] = yb

                def part_c(pr, last=False):
                    p0 = pr * PW
                    for m in range(PW // 128):
                        t0 = p0 + m * 128
                        pso = psp.tile([128, C], f32, tag="pso", bufs=2,
                                       name="pso")
                        if fwd:
                            for cb in range(CB):
                                nc.tensor.matmul(
                                    pso[:],
                                    ypf[cb][:, t0: t0 + 128],
                                    wout[:, cb * C: (cb + 1) * C],
                                    start=(cb == 0), stop=(cb == 3))
                            pstg = wkp.tile([128, C], bf16, tag="pstg",
                                            bufs=3, name="pstg")
                            nc.scalar.copy(pstg[:], pso[:])
                            nc.sync.dma_start(part_d[t0: t0 + 128, :],
                                              pstg[:])
                        else:
                            pstg = wkp.tile([128, C], bf16, tag="pstg",
                                            bufs=3, name="pstg")
                            nc.sync.dma_start(pstg[:],
                                              part_d[t0: t0 + 128, :])
                            nc.tensor.matmul(
                                pso[:], ident[:], pstg[:],
                                start=True, stop=False)
                            for cb in range(CB):
                                nc.tensor.matmul(
                                    pso[:],
                                    ypb_tiles[(pr, cb)][:, m * 128:
                                                        (m + 1) * 128],
                                    wout[:, (4 + cb) * C: (5 + cb) * C],
                                    start=False, stop=(cb == 3))
                            osb = wkp.tile([128, C], bf16, tag="osb",
                                           bufs=2, name="osb")
                            if last and m % 2 == 0:
                                nc.vector.tensor_copy(osb[:], pso[:])
                            else:
                                nc.scalar.copy(osb[:], pso[:])
                            nc.sync.dma_start(out_d[t0: t0 + 128, :],
                                              osb[:])

                for i, pr in enumerate(pairs):
                    part_a(pr, first=(fwd and i == 0))
                    part_b(pr, warm=(i == len(pairs) - 1))
                    part_c(pr, last=(i == len(pairs) - 1))
                    if not fwd:
                        for key in list(ypb_tiles):
                            if key[0] == pr:
                                del ypb_tiles[key]

            run_phase("f")
            run_phase("b")

    return nc


def _host_prep(x, W_rkv, W_out, time_decay, time_first, time_decay_rev,
               time_first_rev):
    bf16 = ml_dtypes.bfloat16
    f32 = np.float32

    Wr = W_rkv.reshape(C, 2, 3, C)
    pieces = {
        "w_rf": Wr[:, 0, 0], "w_kf": Wr[:, 0, 1], "w_vf": Wr[:, 0, 2],
        "w_rb": Wr[:, 1, 0], "w_kb": Wr[:, 1, 1], "w_vb": Wr[:, 1, 2],
    }
    wmaps = {}
    for n, p in pieces.items():
        wmaps[n] = np.ascontiguousarray(
            p.reshape(4, 128, C).transpose(1, 0, 2).reshape(128, 4 * C)
        ).astype(bf16)

    Wo = W_out.reshape(8, 128, C).transpose(1, 0, 2).reshape(128, 8 * C)
    wout = np.ascontiguousarray(Wo).astype(bf16)

    eu_f = np.exp(time_first.astype(np.float64)).reshape(C)
    eu_b = np.exp(time_first_rev.astype(np.float64)).reshape(C)
    dec_f = np.exp(-np.exp(time_decay.astype(np.float64))).reshape(C)
    dec_b = np.exp(-np.exp(time_decay_rev.astype(np.float64))).reshape(C)
    cvec = np.ascontiguousarray(
        np.stack([eu_f, eu_b, dec_f, dec_b], axis=1)
    ).astype(f32)

    ident = np.eye(128, dtype=np.float32).astype(bf16)
    shared = dict(wout=wout, cvec=cvec, ident=ident, **wmaps)
    in_maps = []
    for b in range(B):
        m = dict(shared)
        m["xT"] = np.ascontiguousarray(x[b].T).astype(bf16)
        in_maps.append(m)
    return in_maps


def kernel(x, W_rkv, W_out, time_decay, time_first, time_decay_rev,
           time_first_rev, _trace=False):
    from concourse.bass_utils import run_bass_kernel_spmd

    x = np.asarray(x, dtype=np.float32)
    W_rkv = np.asarray(W_rkv, dtype=np.float32)
    W_out = np.asarray(W_out, dtype=np.float32)
    time_decay = np.asarray(time_decay, dtype=np.float32)
    time_first = np.asarray(time_first, dtype=np.float32)
    time_decay_rev = np.asarray(time_decay_rev, dtype=np.float32)
    time_first_rev = np.asarray(time_first_rev, dtype=np.float32)

    if "nc" not in _CACHE:
        _CACHE["nc"] = _build_nc()
    nc = _CACHE["nc"]

    in_maps = _host_prep(x, W_rkv, W_out, time_decay, time_first,
                         time_decay_rev, time_first_rev)
    res = run_bass_kernel_spmd(
        nc, in_maps, core_ids=list(range(B)), trace=_trace
    )
    _CACHE["last_result"] = res
    out = np.stack([res.results[b]["y"].astype(np.float32) for b in range(B)])
    return out
